# revision 17
# baseline (speedup 1.0000x reference)
"""Trainium2 Bass kernel for CausalSelfAttention (PentaNet-quantized weights).

Reference computation (B=2, T=2048, C=1024, H=16 heads, D=64):
    qkv = x @ quant(w_attn).T ; split q,k,v ; causal softmax attention ;
    out = y @ quant(w_proj).T

Sharding: 8 cores = 2 (batch) x 4 (head groups of 4 heads).  Each core
computes its batch element's attention for its 4 heads plus the partial
output projection over its 256 input channels; the host sums the 4
partials per batch (the w_proj contraction is split across head groups).

Device layout avoids all on-chip transposes:
  - host supplies xT = x[b].T  [C, T]
  - qT,kT computed as [o, t] (weights stationary), v as [t, o]
  - scores computed transposed: ST[j, i] = k_j . q_i  (j = key pos)
  - P = exp(ST/8) with causal masking (block-skip + triangular mask)
  - OT[d, i] = sum_j V[j, d] P[j, i] accumulated in PSUM; an extra
    ones-column in V yields the softmax denominator as OT row 64
  - OT normalized is exactly the lhsT the projection needs.
All matmuls run as float32r (full-rate fp32 replicated mode).
"""

import os
import sys

sys.path.insert(0, "/opt/trn_rl_repo")

import numpy as np

import jax

try:
    jax.config.update("jax_compilation_cache_dir", "/root/.cache/jax_bass_neff")
except Exception:
    pass

import concourse.bass as bass
import concourse.tile as tile
from concourse import bacc, mybir
from concourse.bass_utils import run_bass_kernel_spmd

F32 = mybir.dt.float32
F32R = mybir.dt.float32r

B, T, C = 2, 2048, 1024
H, D = 16, 64
HL = 4                    # heads per core
OL = HL * D               # 256 local output channels
KT = C // 128             # 8 k-tiles over C
TT = T // 128             # 16 t-tiles
NCH = T // 512            # 4 i-chunks of 512
SCALE = 1.0 / 8.0         # 1/sqrt(D)


def r(ap):
    return ap


def build_body(ctx, tc, xT, wq, wk, wv, wp, tri, ztri, tri2, onesd, out):
    nc = tc.nc

    consts = ctx.enter_context(tc.tile_pool(name="consts", bufs=1))
    acts = ctx.enter_context(tc.tile_pool(name="acts", bufs=1))
    pp = ctx.enter_context(tc.tile_pool(name="pp", bufs=3))
    rcp = ctx.enter_context(tc.tile_pool(name="rcp", bufs=2))
    bbp = ctx.enter_context(tc.tile_pool(name="bbp", bufs=2))
    obp = ctx.enter_context(tc.tile_pool(name="obp", bufs=3))
    ps_mm = ctx.enter_context(tc.tile_pool(name="ps_mm", bufs=2, space="PSUM"))
    ps_pj = ctx.enter_context(tc.tile_pool(name="ps_pj", bufs=2, space="PSUM"))
    ps_ot = ctx.enter_context(tc.tile_pool(name="ps_ot", bufs=2, space="PSUM"))

    # ---- load inputs to SBUF ----
    # weights first, then xT by column chunk so qkv matmuls start early
    wq_sb = consts.tile([128, KT * OL], F32R)
    wk_sb = consts.tile([128, KT * OL], F32R)
    wv_sb = consts.tile([128, KT * OL], F32R)
    xT_sb = consts.tile([128, KT * T], F32R)

    def load_w(w_sb, w_d):
        # one DMA: DRAM [KT*128, OL] -> SBUF [128, KT, OL]
        nc.sync.dma_start(
            w_sb[:].rearrange("p (k o) -> p k o", k=KT),
            w_d.rearrange("(k p) o -> p k o", k=KT))

    def load_x(n):
        for k in range(KT):
            nc.sync.dma_start(
                xT_sb[:, k * T + n * 512: k * T + (n + 1) * 512],
                xT[k * 128:(k + 1) * 128, n * 512:(n + 1) * 512])

    load_w(wq_sb, wq)
    load_w(wv_sb, wv)
    load_x(0)
    load_w(wk_sb, wk)
    for n in range(1, NCH):
        load_x(n)
    tri_sb = consts.tile([128, 128], F32R)
    nc.sync.dma_start(tri_sb[:], tri[:, 0:128])
    ztri_sb = consts.tile([128, 256], F32R)
    nc.sync.dma_start(ztri_sb[:], ztri[:, :])
    tri_sb2 = consts.tile([128, 256], F32R)
    nc.sync.dma_start(tri_sb2[:], tri2[:, :])
    wp_sb = consts.tile([128, 2 * C], F32R)
    nc.sync.dma_start(
        wp_sb[:].rearrange("p (k o) -> p k o", k=2),
        wp.rearrange("(k p) o -> p k o", k=2))

    q_sb = acts.tile([128, 2 * T], F32R)
    k_sb = acts.tile([128, 2 * T], F32R)
    v_sb = acts.tile([128, TT * HL * (D + 1)], F32R)
    yt_sb = acts.tile([128, 2 * T], F32R)
    # ones column (index D) of every [t-tile, head] V block
    v_ones = v_sb[:].rearrange("p (g c) -> p g c", c=D + 1)[:, :, D]
    nc.sync.dma_start(v_ones, onesd[:, :])

    # ---- emission units ----
    # qk_unit / v_unit / proj_unit: one PSUM accumulation group each (PE filler
    # work).  attn blocks: the ST->exp->OT chain that keeps ACT busy.
    def qk_unit(n, which, m):
        w_sb, dst = (wq_sb, q_sb) if which == 0 else (wk_sb, k_sb)
        ps = ps_pj.tile([128, 512], F32, tag="pj")
        for k in range(KT):
            nc.tensor.matmul(
                ps[:],
                r(w_sb[:, k * OL + m * 128: k * OL + (m + 1) * 128]),
                r(xT_sb[:, k * T + n * 512: k * T + (n + 1) * 512]),
                start=(k == 0), stop=(k == KT - 1),
            )
        nc.vector.tensor_copy(dst[:, m * T + n * 512: m * T + (n + 1) * 512], ps[:])

    def v_unit(t):
        ps = ps_pj.tile([128, OL], F32, tag="pj")
        for k in range(KT):
            nc.tensor.matmul(
                ps[:],
                r(xT_sb[:, k * T + t * 128: k * T + (t + 1) * 128]),
                r(wv_sb[:, k * OL:(k + 1) * OL]),
                start=(k == 0), stop=(k == KT - 1),
            )
        dst = v_sb[:, t * HL * (D + 1): (t + 1) * HL * (D + 1)]
        dst = dst.rearrange("p (h c) -> p h c", h=HL)[:, :, 0:D]
        nc.vector.tensor_copy(dst, ps[:].rearrange("p (h c) -> p h c", h=HL))

    def proj_unit(t, n2):
        # n2 == 0 computes cols [0:512] into ob; n2 == 1 computes [512:1024]
        # and issues the single merged store for the whole t-tile row block.
        ps = ps_pj.tile([128, 512], F32, tag="pj")
        for kk in range(2):
            nc.tensor.matmul(
                ps[:],
                r(yt_sb[:, kk * T + t * 128: kk * T + (t + 1) * 128]),
                r(wp_sb[:, kk * C + n2 * 512: kk * C + (n2 + 1) * 512]),
                start=(kk == 0), stop=(kk == 1),
            )
        ob = _ob_state.get(t)
        if ob is None:
            ob = obp.tile([128, 1024], F32, tag="ob", name=f"ob_{t}")
            _ob_state[t] = ob
        if n2 == 0:
            nc.vector.tensor_copy(ob[:, 0:512], ps[:])
        else:
            nc.scalar.copy(ob[:, 512:1024], ps[:])
            nc.sync.dma_start(out[t * 128:(t + 1) * 128, :], ob[:])
            del _ob_state[t]

    _ob_state = {}

    def qkv_units(n):
        return ([(lambda n=n, w=w, m=m: qk_unit(n, w, m)) for w in range(2) for m in range(2)]
                + [(lambda t=t: v_unit(t)) for t in range(4 * n, 4 * n + 4)])

    def proj_units(ic):
        return [(lambda t=t, n2=n2: proj_unit(t, n2))
                for t in range(4 * ic, 4 * ic + 4) for n2 in range(2)]

    # attention chunk as a list of work-item callables; the chunk's ps_o
    # tiles live across its items.  Full blocks (tj < 4*ic) are processed in
    # pairs sharing one [128,1024] PSUM tile and one exp; diagonal blocks are
    # singles (d=3 widened to 256 so the fp32r matmuls stay full-rate, with
    # the extra columns masked to zero).
    def attn_blocks(ic):
        items = []
        for h in range(HL):
            pb = 64 * (h % 2)
            mo = (h // 2) * T
            state = {}

            def ot_mm(tj, rhs, cs, h=h, ic=ic, state=state):
                vh = v_sb[:, (tj * HL + h) * (D + 1):(tj * HL + h + 1) * (D + 1)]
                nc.tensor.matmul(
                    state["ps_o"][:, cs:512],
                    r(vh), r(rhs),
                    start=(tj == 0), stop=(tj == 4 * ic + 3),
                    skip_group_check=True,
                )

            def open_ps(h=h, ic=ic, state=state):
                ps_o = ps_ot.tile([D + 1, 512], F32, tag="ot", name=f"ps_o_{ic}_{h}")
                state["ps_o"] = ps_o

            def full_pair(tja, h=h, pb=pb, mo=mo, ic=ic, state=state, open_ps=open_ps, ot_mm=ot_mm):
                if tja == 0:
                    open_ps()
                qh = q_sb[pb:pb + 64, mo:mo + T]
                kh = k_sb[pb:pb + 64, mo:mo + T]
                ps_s = ps_mm.tile([128, 1024], F32, tag="mm", name=f"ps_s_{ic}_{h}")
                for j in range(2):
                    nc.tensor.matmul(
                        ps_s[:, j * 512:(j + 1) * 512],
                        r(kh[:, (tja + j) * 128:(tja + j + 1) * 128]),
                        r(qh[:, ic * 512:(ic + 1) * 512]),
                        start=True, stop=True,
                        skip_group_check=True,
                    )
                p_t = pp.tile([128, 1024], F32R, tag="p", name=f"p_t_{ic}_{h}")
                nc.scalar.activation(p_t[:], ps_s[:], mybir.ActivationFunctionType.Exp,
                                     scale=SCALE)
                for j in range(2):
                    ot_mm(tja + j, p_t[:, j * 512:(j + 1) * 512], 0)

            def diag_pair(da, h=h, pb=pb, mo=mo, ic=ic, state=state, open_ps=open_ps, ot_mm=ot_mm):
                # da=0: blocks d=0 (w 512, cs 0) + d=1 (w 384, cs 128) in one
                # [128, 896] tile/exp.  da=2: d=2 + d=3 (both w 256, cs 256)
                # in one [128, 512] tile/exp.  d=3 uses the zero|tri mask.
                if 4 * ic + da == 0:
                    open_ps()
                qh = q_sb[pb:pb + 64, mo:mo + T]
                kh = k_sb[pb:pb + 64, mo:mo + T]
                if da == 0:
                    widths, css = (512, 384), (0, 128)
                else:
                    widths, css = (256, 256), (256, 256)
                tot_w = widths[0] + widths[1]
                ps_s = ps_mm.tile([128, tot_w], F32, tag="mm", name=f"ps_d_{ic}_{h}")
                off = 0
                for j in range(2):
                    d = da + j
                    tj = 4 * ic + d
                    nc.tensor.matmul(
                        ps_s[:, off:off + widths[j]],
                        r(kh[:, tj * 128:(tj + 1) * 128]),
                        r(qh[:, ic * 512 + css[j]:(ic + 1) * 512]),
                        start=True, stop=True,
                        skip_group_check=True,
                    )
                    off += widths[j]
                p_t = pp.tile([128, tot_w], F32R, tag="p", name=f"p_d_{ic}_{h}")
                nc.scalar.activation(p_t[:], ps_s[:], mybir.ActivationFunctionType.Exp,
                                     scale=SCALE)
                if da == 0:
                    nc.vector.tensor_mul(p_t[:, 0:128], p_t[:, 0:128], tri_sb[:])
                    nc.vector.tensor_mul(p_t[:, 512:640], p_t[:, 512:640], tri_sb[:])
                else:
                    nc.vector.tensor_mul(p_t[:, 0:256], p_t[:, 0:256], tri_sb2[:])
                    nc.vector.tensor_mul(p_t[:, 256:512], p_t[:, 256:512], ztri_sb[:])
                off = 0
                for j in range(2):
                    d = da + j
                    ot_mm(4 * ic + d, p_t[:, off:off + widths[j]], css[j])
                    off += widths[j]
                if da == 2:
                    # normalize: yt[o, i] = OT[d, i] / s[i]
                    ps_o = state["ps_o"]
                    rc = rcp.tile([1, 512], F32R, tag="rc")
                    with nc.allow_low_precision(reason="fp32r ~ fp32 denom"):
                        nc.vector.reciprocal(rc[:], ps_o[D:D + 1, :])
                    bb = bbp.tile([64, 512], F32R, tag="bb")
                    nc.gpsimd.partition_broadcast(bb[:], rc[:])
                    nc.vector.tensor_mul(
                        yt_sb[pb:pb + 64, mo + ic * 512: mo + (ic + 1) * 512],
                        ps_o[0:D, :], bb[:],
                    )

            for tja in range(0, 4 * ic, 2):
                items.append(lambda tja=tja, f=full_pair: f(tja))
            for da in (0, 2):
                items.append(lambda da=da, f=diag_pair: f(da))
        return items

    def emit_interleaved(blocks, fillers):
        """Emit attention blocks with filler units spread evenly between."""
        nf = len(fillers)
        nb = len(blocks)
        fi = 0
        for i, blk in enumerate(blocks):
            blk()
            want = (i + 1) * nf // nb
            while fi < want:
                fillers[fi]()
                fi += 1
        while fi < nf:
            fillers[fi]()
            fi += 1

    # schedule: qkv(0) first; attention chunk ic interleaves qkv(ic+1) and
    # proj(ic-1); proj(3) trails.
    for u in qkv_units(0):
        u()
    for ic in range(NCH):
        fill = []
        if ic + 1 < NCH:
            fill += qkv_units(ic + 1)
        if ic - 1 >= 0:
            fill += proj_units(ic - 1)
        emit_interleaved(attn_blocks(ic), fill)
    for u in proj_units(NCH - 1):
        u()


def build_program(reps=1):
    from contextlib import ExitStack

    nc = bacc.Bacc("TRN2", target_bir_lowering=False, debug=False)
    xT = nc.dram_tensor("xT", [C, T], F32R, kind="ExternalInput").ap()
    wq = nc.dram_tensor("wq", [C, OL], F32R, kind="ExternalInput").ap()
    wk = nc.dram_tensor("wk", [C, OL], F32R, kind="ExternalInput").ap()
    wv = nc.dram_tensor("wv", [C, OL], F32R, kind="ExternalInput").ap()
    wp = nc.dram_tensor("wp", [OL, C], F32R, kind="ExternalInput").ap()
    tri = nc.dram_tensor("tri", [128, 128], F32R, kind="ExternalInput").ap()
    ztri = nc.dram_tensor("ztri", [128, 256], F32R, kind="ExternalInput").ap()
    tri2 = nc.dram_tensor("tri2", [128, 256], F32R, kind="ExternalInput").ap()
    onesd = nc.dram_tensor("onesd", [128, TT * HL], F32R, kind="ExternalInput").ap()
    out = nc.dram_tensor("out", [T, C], F32, kind="ExternalOutput").ap()

    with tile.TileContext(nc) as tc:
        for _ in range(reps):
            with ExitStack() as ctx:
                build_body(ctx, tc, xT, wq, wk, wv, wp, tri, ztri, tri2, onesd, out)
    nc.compile()
    return nc


def quant_weight_np(w):
    scale = max(np.mean(np.abs(w), dtype=np.float32), np.float32(1e-8))
    return (np.clip(np.round(w / scale), -2.0, 2.0) * scale).astype(np.float32)


def make_in_maps(x, w_attn, w_proj):
    wq_f = quant_weight_np(w_attn)
    wp_f = quant_weight_np(w_proj)
    tri = np.triu(np.ones((128, 128), dtype=np.float32))
    ztri = np.concatenate([np.zeros((128, 128), dtype=np.float32), tri], axis=1)
    tri2 = np.concatenate([tri, np.ones((128, 128), dtype=np.float32)], axis=1)
    in_maps = []
    for core in range(8):
        b, g = divmod(core, 4)
        sl = slice(g * OL, (g + 1) * OL)
        in_maps.append({
            "xT": np.ascontiguousarray(x[b].T),
            "wq": np.ascontiguousarray(wq_f[0 * C:1 * C][sl].T),
            "wk": np.ascontiguousarray(wq_f[1 * C:2 * C][sl].T),
            "wv": np.ascontiguousarray(wq_f[2 * C:3 * C][sl].T),
            "wp": np.ascontiguousarray(wp_f[:, sl].T),
            "tri": tri,
            "ztri": ztri,
            "tri2": tri2,
            "onesd": np.ones((128, TT * HL), dtype=np.float32),
        })
    return in_maps


_CACHED_NC = None


def kernel(x, w_attn, w_proj):
    global _CACHED_NC
    if _CACHED_NC is None:
        _CACHED_NC = build_program()
    in_maps = make_in_maps(np.asarray(x, dtype=np.float32),
                           np.asarray(w_attn, dtype=np.float32),
                           np.asarray(w_proj, dtype=np.float32))
    res = run_bass_kernel_spmd(_CACHED_NC, in_maps, list(range(8)))
    out = np.zeros((B, T, C), dtype=np.float32)
    for core in range(8):
        b = core // 4
        out[b] += res.results[core]["out"]
    return out


# revision 20
# speedup vs baseline: 3.8565x; 3.8565x over previous
"""Trainium2 Bass kernel for CausalSelfAttention (PentaNet-quantized weights).

Reference computation (B=2, T=2048, C=1024, H=16 heads, D=64):
    qkv = x @ quant(w_attn).T ; split q,k,v ; causal softmax attention ;
    out = y @ quant(w_proj).T

Sharding: 8 cores = 2 (batch) x 4 (head groups of 4 heads).  Each core
computes its batch element's attention for its 4 heads plus the partial
output projection over its 256 input channels; the host sums the 4
partials per batch (the w_proj contraction is split across head groups).

Device layout avoids all on-chip transposes:
  - host supplies xT = x[b].T  [C, T]
  - qT,kT computed as [o, t] (weights stationary), v as [t, o]
  - scores computed transposed: ST[j, i] = k_j . q_i  (j = key pos)
  - P = exp(ST/8) with causal masking (block-skip + triangular mask)
  - OT[d, i] = sum_j V[j, d] P[j, i] accumulated in PSUM; an extra
    ones-column in V yields the softmax denominator as OT row 64
  - OT normalized is exactly the lhsT the projection needs.
All matmuls run as float32r (full-rate fp32 replicated mode).
"""

import os
import sys

sys.path.insert(0, "/opt/trn_rl_repo")

import numpy as np
import ml_dtypes

import jax

try:
    jax.config.update("jax_compilation_cache_dir", "/root/.cache/jax_bass_neff")
except Exception:
    pass

import concourse.bass as bass
import concourse.tile as tile
from concourse import bacc, mybir
from concourse.bass_utils import run_bass_kernel_spmd

F32 = mybir.dt.float32
F32R = mybir.dt.float32r
BF16 = mybir.dt.bfloat16

B, T, C = 2, 2048, 1024
H, D = 16, 64
HL = 4                    # heads per core
OL = HL * D               # 256 local output channels
KT = C // 128             # 8 k-tiles over C
TT = T // 128             # 16 t-tiles
NCH = T // 512            # 4 i-chunks of 512
SCALE = 1.0 / 8.0         # 1/sqrt(D)


def r(ap):
    return ap


def build_body(ctx, tc, xT, wq, wk, wv, wp, tri, ztri, tri2, onesd, out):
    nc = tc.nc

    consts = ctx.enter_context(tc.tile_pool(name="consts", bufs=1))
    acts = ctx.enter_context(tc.tile_pool(name="acts", bufs=1))
    pp = ctx.enter_context(tc.tile_pool(name="pp", bufs=3))
    rcp = ctx.enter_context(tc.tile_pool(name="rcp", bufs=2))
    bbp = ctx.enter_context(tc.tile_pool(name="bbp", bufs=2))
    obp = ctx.enter_context(tc.tile_pool(name="obp", bufs=3))
    ps_mm = ctx.enter_context(tc.tile_pool(name="ps_mm", bufs=2, space="PSUM"))
    ps_pj = ctx.enter_context(tc.tile_pool(name="ps_pj", bufs=2, space="PSUM"))
    ps_ot = ctx.enter_context(tc.tile_pool(name="ps_ot", bufs=2, space="PSUM"))

    # ---- load inputs to SBUF ----
    # weights first, then xT by column chunk so qkv matmuls start early
    wq_sb = consts.tile([128, KT * OL], BF16)
    wk_sb = consts.tile([128, KT * OL], BF16)
    wv_sb = consts.tile([128, KT * OL], BF16)
    xT_sb = consts.tile([128, KT * T], BF16)

    def load_w(w_sb, w_d):
        # one DMA: DRAM [KT*128, OL] -> SBUF [128, KT, OL]
        nc.sync.dma_start(
            w_sb[:].rearrange("p (k o) -> p k o", k=KT),
            w_d.rearrange("(k p) o -> p k o", k=KT))

    def load_x(n):
        for k in range(KT):
            nc.sync.dma_start(
                xT_sb[:, k * T + n * 512: k * T + (n + 1) * 512],
                xT[k * 128:(k + 1) * 128, n * 512:(n + 1) * 512])

    load_w(wq_sb, wq)
    load_w(wv_sb, wv)
    load_x(0)
    load_w(wk_sb, wk)
    for n in range(1, NCH):
        load_x(n)
    tri_sb = consts.tile([128, 128], BF16)
    nc.sync.dma_start(tri_sb[:], tri[:, 0:128])
    ztri_sb = consts.tile([128, 256], BF16)
    nc.sync.dma_start(ztri_sb[:], ztri[:, :])
    tri_sb2 = consts.tile([128, 256], BF16)
    nc.sync.dma_start(tri_sb2[:], tri2[:, :])
    wp_sb = consts.tile([128, 2 * C], BF16)
    nc.sync.dma_start(
        wp_sb[:].rearrange("p (k o) -> p k o", k=2),
        wp.rearrange("(k p) o -> p k o", k=2))

    q_sb = acts.tile([128, 2 * T], BF16)
    k_sb = acts.tile([128, 2 * T], BF16)
    v_sb = acts.tile([128, TT * HL * (D + 1)], BF16)
    yt_sb = acts.tile([128, 2 * T], BF16)
    # ones column (index D) of every [t-tile, head] V block
    v_ones = v_sb[:].rearrange("p (g c) -> p g c", c=D + 1)[:, :, D]
    nc.sync.dma_start(v_ones, onesd[:, :])

    # ---- emission units ----
    # qk_unit / v_unit / proj_unit: one PSUM accumulation group each (PE filler
    # work).  attn blocks: the ST->exp->OT chain that keeps ACT busy.
    def qk_unit(n, which, m):
        w_sb, dst = (wq_sb, q_sb) if which == 0 else (wk_sb, k_sb)
        ps = ps_pj.tile([128, 512], F32, tag="pj")
        for k in range(KT):
            nc.tensor.matmul(
                ps[:],
                r(w_sb[:, k * OL + m * 128: k * OL + (m + 1) * 128]),
                r(xT_sb[:, k * T + n * 512: k * T + (n + 1) * 512]),
                start=(k == 0), stop=(k == KT - 1),
            )
        nc.vector.tensor_copy(dst[:, m * T + n * 512: m * T + (n + 1) * 512], ps[:])

    def v_unit(t):
        ps = ps_pj.tile([128, OL], F32, tag="pj")
        for k in range(KT):
            nc.tensor.matmul(
                ps[:],
                r(xT_sb[:, k * T + t * 128: k * T + (t + 1) * 128]),
                r(wv_sb[:, k * OL:(k + 1) * OL]),
                start=(k == 0), stop=(k == KT - 1),
            )
        dst = v_sb[:, t * HL * (D + 1): (t + 1) * HL * (D + 1)]
        dst = dst.rearrange("p (h c) -> p h c", h=HL)[:, :, 0:D]
        nc.vector.tensor_copy(dst, ps[:].rearrange("p (h c) -> p h c", h=HL))

    def proj_unit(t, n2):
        # n2 == 0 computes cols [0:512] into ob; n2 == 1 computes [512:1024]
        # and issues the single merged store for the whole t-tile row block.
        ps = ps_pj.tile([128, 512], F32, tag="pj")
        for kk in range(2):
            nc.tensor.matmul(
                ps[:],
                r(yt_sb[:, kk * T + t * 128: kk * T + (t + 1) * 128]),
                r(wp_sb[:, kk * C + n2 * 512: kk * C + (n2 + 1) * 512]),
                start=(kk == 0), stop=(kk == 1),
            )
        ob = _ob_state.get(t)
        if ob is None:
            ob = obp.tile([128, 1024], F32, tag="ob", name=f"ob_{t}")
            _ob_state[t] = ob
        if n2 == 0:
            nc.vector.tensor_copy(ob[:, 0:512], ps[:])
        else:
            nc.scalar.copy(ob[:, 512:1024], ps[:])
            nc.sync.dma_start(out[t * 128:(t + 1) * 128, :], ob[:])
            del _ob_state[t]

    _ob_state = {}

    def qkv_units(n):
        return ([(lambda n=n, w=w, m=m: qk_unit(n, w, m)) for w in range(2) for m in range(2)]
                + [(lambda t=t: v_unit(t)) for t in range(4 * n, 4 * n + 4)])

    def proj_units(ic):
        return [(lambda t=t, n2=n2: proj_unit(t, n2))
                for t in range(4 * ic, 4 * ic + 4) for n2 in range(2)]

    # attention chunk as a list of work-item callables; the chunk's ps_o
    # tiles live across its items.  Full blocks (tj < 4*ic) are processed in
    # pairs sharing one [128,1024] PSUM tile and one exp; diagonal blocks are
    # singles (d=3 widened to 256 so the fp32r matmuls stay full-rate, with
    # the extra columns masked to zero).
    def attn_blocks(ic):
        items = []
        for h in range(HL):
            pb = 64 * (h % 2)
            mo = (h // 2) * T
            state = {}

            def ot_mm(tj, rhs, cs, h=h, ic=ic, state=state):
                vh = v_sb[:, (tj * HL + h) * (D + 1):(tj * HL + h + 1) * (D + 1)]
                nc.tensor.matmul(
                    state["ps_o"][:, cs:512],
                    r(vh), r(rhs),
                    start=(tj == 0), stop=(tj == 4 * ic + 3),
                    skip_group_check=True,
                )

            def open_ps(h=h, ic=ic, state=state):
                ps_o = ps_ot.tile([D + 1, 512], F32, tag="ot", name=f"ps_o_{ic}_{h}")
                state["ps_o"] = ps_o

            def full_pair(tja, h=h, pb=pb, mo=mo, ic=ic, state=state, open_ps=open_ps, ot_mm=ot_mm):
                if tja == 0:
                    open_ps()
                qh = q_sb[pb:pb + 64, mo:mo + T]
                kh = k_sb[pb:pb + 64, mo:mo + T]
                ps_s = ps_mm.tile([128, 1024], F32, tag="mm", name=f"ps_s_{ic}_{h}")
                for j in range(2):
                    nc.tensor.matmul(
                        ps_s[:, j * 512:(j + 1) * 512],
                        r(kh[:, (tja + j) * 128:(tja + j + 1) * 128]),
                        r(qh[:, ic * 512:(ic + 1) * 512]),
                        start=True, stop=True,
                        skip_group_check=True,
                    )
                p_t = pp.tile([128, 1024], BF16, tag="p", name=f"p_t_{ic}_{h}")
                nc.scalar.activation(p_t[:], ps_s[:], mybir.ActivationFunctionType.Exp,
                                     scale=SCALE)
                for j in range(2):
                    ot_mm(tja + j, p_t[:, j * 512:(j + 1) * 512], 0)

            def diag_pair(da, h=h, pb=pb, mo=mo, ic=ic, state=state, open_ps=open_ps, ot_mm=ot_mm):
                # da=0: blocks d=0 (w 512, cs 0) + d=1 (w 384, cs 128) in one
                # [128, 896] tile/exp.  da=2: d=2 + d=3 (both w 256, cs 256)
                # in one [128, 512] tile/exp.  d=3 uses the zero|tri mask.
                if 4 * ic + da == 0:
                    open_ps()
                qh = q_sb[pb:pb + 64, mo:mo + T]
                kh = k_sb[pb:pb + 64, mo:mo + T]
                if da == 0:
                    widths, css = (512, 384), (0, 128)
                else:
                    widths, css = (256, 256), (256, 256)
                tot_w = widths[0] + widths[1]
                ps_s = ps_mm.tile([128, tot_w], F32, tag="mm", name=f"ps_d_{ic}_{h}")
                off = 0
                for j in range(2):
                    d = da + j
                    tj = 4 * ic + d
                    nc.tensor.matmul(
                        ps_s[:, off:off + widths[j]],
                        r(kh[:, tj * 128:(tj + 1) * 128]),
                        r(qh[:, ic * 512 + css[j]:(ic + 1) * 512]),
                        start=True, stop=True,
                        skip_group_check=True,
                    )
                    off += widths[j]
                p_t = pp.tile([128, tot_w], BF16, tag="p", name=f"p_d_{ic}_{h}")
                nc.scalar.activation(p_t[:], ps_s[:], mybir.ActivationFunctionType.Exp,
                                     scale=SCALE)
                if da == 0:
                    nc.vector.tensor_mul(p_t[:, 0:128], p_t[:, 0:128], tri_sb[:])
                    nc.vector.tensor_mul(p_t[:, 512:640], p_t[:, 512:640], tri_sb[:])
                else:
                    nc.vector.tensor_mul(p_t[:, 0:256], p_t[:, 0:256], tri_sb2[:])
                    nc.vector.tensor_mul(p_t[:, 256:512], p_t[:, 256:512], ztri_sb[:])
                off = 0
                for j in range(2):
                    d = da + j
                    ot_mm(4 * ic + d, p_t[:, off:off + widths[j]], css[j])
                    off += widths[j]
                if da == 2:
                    # normalize: yt[o, i] = OT[d, i] / s[i]
                    ps_o = state["ps_o"]
                    rc = rcp.tile([1, 512], F32R, tag="rc")
                    with nc.allow_low_precision(reason="fp32r ~ fp32 denom"):
                        nc.vector.reciprocal(rc[:], ps_o[D:D + 1, :])
                    bb = bbp.tile([64, 512], F32R, tag="bb")
                    nc.gpsimd.partition_broadcast(bb[:], rc[:])
                    nc.vector.tensor_mul(
                        yt_sb[pb:pb + 64, mo + ic * 512: mo + (ic + 1) * 512],
                        ps_o[0:D, :], bb[:],
                    )

            for tja in range(0, 4 * ic, 2):
                items.append(lambda tja=tja, f=full_pair: f(tja))
            for da in (0, 2):
                items.append(lambda da=da, f=diag_pair: f(da))
        return items

    def emit_interleaved(blocks, fillers):
        """Emit attention blocks with filler units spread evenly between."""
        nf = len(fillers)
        nb = len(blocks)
        fi = 0
        for i, blk in enumerate(blocks):
            blk()
            want = (i + 1) * nf // nb
            while fi < want:
                fillers[fi]()
                fi += 1
        while fi < nf:
            fillers[fi]()
            fi += 1

    # schedule: qkv(0) first; attention chunk ic interleaves qkv(ic+1) and
    # proj(ic-1); proj(3) trails.
    for u in qkv_units(0):
        u()
    for ic in range(NCH):
        fill = []
        if ic + 1 < NCH:
            fill += qkv_units(ic + 1)
        if ic - 1 >= 0:
            fill += proj_units(ic - 1)
        emit_interleaved(attn_blocks(ic), fill)
    for u in proj_units(NCH - 1):
        u()


def build_program(reps=1):
    from contextlib import ExitStack

    nc = bacc.Bacc("TRN2", target_bir_lowering=False, debug=False)
    xT = nc.dram_tensor("xT", [C, T], BF16, kind="ExternalInput").ap()
    wq = nc.dram_tensor("wq", [C, OL], BF16, kind="ExternalInput").ap()
    wk = nc.dram_tensor("wk", [C, OL], BF16, kind="ExternalInput").ap()
    wv = nc.dram_tensor("wv", [C, OL], BF16, kind="ExternalInput").ap()
    wp = nc.dram_tensor("wp", [OL, C], BF16, kind="ExternalInput").ap()
    tri = nc.dram_tensor("tri", [128, 128], BF16, kind="ExternalInput").ap()
    ztri = nc.dram_tensor("ztri", [128, 256], BF16, kind="ExternalInput").ap()
    tri2 = nc.dram_tensor("tri2", [128, 256], BF16, kind="ExternalInput").ap()
    onesd = nc.dram_tensor("onesd", [128, TT * HL], BF16, kind="ExternalInput").ap()
    out = nc.dram_tensor("out", [T, C], F32, kind="ExternalOutput").ap()

    with tile.TileContext(nc) as tc:
        for _ in range(reps):
            with ExitStack() as ctx:
                build_body(ctx, tc, xT, wq, wk, wv, wp, tri, ztri, tri2, onesd, out)
    nc.compile()
    return nc


def quant_weight_np(w):
    scale = max(np.mean(np.abs(w), dtype=np.float32), np.float32(1e-8))
    return (np.clip(np.round(w / scale), -2.0, 2.0) * scale).astype(np.float32)


def make_in_maps(x, w_attn, w_proj):
    wq_f = quant_weight_np(w_attn)
    wp_f = quant_weight_np(w_proj)
    tri = np.triu(np.ones((128, 128), dtype=np.float32))
    ztri = np.concatenate([np.zeros((128, 128), dtype=np.float32), tri], axis=1)
    tri2 = np.concatenate([tri, np.ones((128, 128), dtype=np.float32)], axis=1)
    in_maps = []
    for core in range(8):
        b, g = divmod(core, 4)
        sl = slice(g * OL, (g + 1) * OL)
        in_maps.append({
            "xT": np.ascontiguousarray(x[b].T).astype(ml_dtypes.bfloat16),
            "wq": np.ascontiguousarray(wq_f[0 * C:1 * C][sl].T).astype(ml_dtypes.bfloat16),
            "wk": np.ascontiguousarray(wq_f[1 * C:2 * C][sl].T).astype(ml_dtypes.bfloat16),
            "wv": np.ascontiguousarray(wq_f[2 * C:3 * C][sl].T).astype(ml_dtypes.bfloat16),
            "wp": np.ascontiguousarray(wp_f[:, sl].T).astype(ml_dtypes.bfloat16),
            "tri": tri.astype(ml_dtypes.bfloat16),
            "ztri": ztri.astype(ml_dtypes.bfloat16),
            "tri2": tri2.astype(ml_dtypes.bfloat16),
            "onesd": np.ones((128, TT * HL), dtype=ml_dtypes.bfloat16),
        })
    return in_maps


_CACHED_NC = None


def kernel(x, w_attn, w_proj):
    global _CACHED_NC
    if _CACHED_NC is None:
        _CACHED_NC = build_program()
    in_maps = make_in_maps(np.asarray(x, dtype=np.float32),
                           np.asarray(w_attn, dtype=np.float32),
                           np.asarray(w_proj, dtype=np.float32))
    res = run_bass_kernel_spmd(_CACHED_NC, in_maps, list(range(8)))
    out = np.zeros((B, T, C), dtype=np.float32)
    for core in range(8):
        b = core // 4
        out[b] += res.results[core]["out"]
    return out
